# revision 5
# baseline (speedup 1.0000x reference)
"""LoRA Linear (T=8192, D_in=D_out=4096, r=16) on 8 TRN2 NeuronCores.

out = x @ W^T + b + (32/16) * ((x_bf16 @ A^T) @ B^T)

Strategy: data-parallel over the 8192-token axis (1024 tokens/core).
Host pre-transposes operands so the contraction dim d lands on SBUF
partitions with perfectly contiguous DMA:
  xT  [4096, 1024] fp32  (per-core shard, SBUF-resident, stationary operand)
  WT  [4096, 4096] fp32  (replicated, streamed once per core, moving operand)
Base matmul runs as float32r (fp32 truncated to ~FP22 in the PE) which is
full-rate when the moving free dim >= 256 -- vs 4x slower true fp32.
LoRA: lora1^T = A @ x^T computed first (fp32r, rank-16 output), rounded to
bf16 (matching the reference's bf16 intermediate), then the rank-16
expansion matmul (bf16) seeds each PSUM accumulation group before the 32
base-matmul accumulations; bias is added on the PSUM->SBUF copy (DVE).
LoRA scaling (32/16 = 2.0) is folded into B^T on the host (exact in bf16).
"""

import numpy as np

try:
    import concourse  # noqa: F401
except ImportError:  # pragma: no cover
    import sys

    sys.path.insert(0, "/opt/trn_rl_repo")

from concourse import bacc, mybir, tile
from concourse.bass_utils import run_bass_kernel_spmd

N_CORES = 8
T, D_IN, D_OUT, R = 8192, 4096, 4096, 16
TPC = T // N_CORES  # 1024 tokens per core
N_DC = D_IN // 128  # 32 contraction chunks of 128
OC = 512  # output-column chunk (one PSUM bank of fp32)
N_OC = D_OUT // OC  # 8
N_TC = TPC // 128  # 8 token tiles of 128

f32 = mybir.dt.float32
f32r = mybir.dt.float32r
bf16 = mybir.dt.bfloat16

_NC_CACHE = {}


def build_nc(reps=1):
    nc = bacc.Bacc(
        "TRN2", target_bir_lowering=False, debug=False, num_devices=N_CORES
    )
    xT = nc.dram_tensor("xT", [D_IN, TPC], f32r, kind="ExternalInput").ap()
    WT = nc.dram_tensor("WT", [D_IN, D_OUT], f32r, kind="ExternalInput").ap()
    AT = nc.dram_tensor("AT", [D_IN, R], f32r, kind="ExternalInput").ap()
    BT = nc.dram_tensor("BT", [R, D_OUT], bf16, kind="ExternalInput").ap()
    bias = nc.dram_tensor("bias", [128, D_OUT], f32, kind="ExternalInput").ap()
    out = nc.dram_tensor("out", [TPC, D_OUT], f32, kind="ExternalOutput").ap()

    with tile.TileContext(nc) as tc:
        with (
            tc.tile_pool(name="persist", bufs=1) as persist,
            tc.tile_pool(name="xpool", bufs=N_DC) as xpool,
            tc.tile_pool(name="wpool", bufs=4) as wpool,
            tc.tile_pool(name="opool", bufs=6) as opool,
            tc.tile_pool(name="pspool", bufs=8, space="PSUM") as pspool,
        ):
          for _rep in range(reps):
            at_sb = persist.tile([128, N_DC * R], f32r, tag="at")
            bt_sb = persist.tile([R, D_OUT], bf16, tag="bt")
            bias_sb = persist.tile([128, D_OUT], f32, tag="bias")
            lora1_sb = persist.tile([R, TPC], bf16, tag="lora1")

            nc.sync.dma_start(out=bias_sb[:], in_=bias[:])
            nc.sync.dma_start(out=bt_sb[:], in_=BT[:])
            for dc in range(N_DC):
                nc.sync.dma_start(
                    out=at_sb[:, dc * R : (dc + 1) * R],
                    in_=AT[dc * 128 : (dc + 1) * 128, :],
                )

            xt_tiles = []
            for dc in range(N_DC):
                xt = xpool.tile([128, TPC], f32r, tag="xt")
                nc.sync.dma_start(
                    out=xt[:], in_=xT[dc * 128 : (dc + 1) * 128, :]
                )
                xt_tiles.append(xt)

            # Phase 1: lora1T[r, t] = sum_d A[r, d] * x[t, d]  (fp32r),
            # rounded to bf16 like the reference's bf16 einsum output.
            for th in range(TPC // OC):
                ps_l = pspool.tile([R, OC], f32, tag="ps")
                for dc in range(N_DC):
                    nc.tensor.matmul(
                        ps_l[:],
                        at_sb[:, dc * R : (dc + 1) * R],
                        xt_tiles[dc][:, th * OC : (th + 1) * OC],
                        start=(dc == 0),
                        stop=(dc == N_DC - 1),
                    )
                nc.vector.tensor_copy(
                    lora1_sb[:, th * OC : (th + 1) * OC], ps_l[:]
                )

            # Phase 2: out[t, o] = lora2 + sum_d x[t, d] W[o, d] + bias
            for oc in range(N_OC):
                osl = slice(oc * OC, (oc + 1) * OC)
                ps_tiles = [
                    pspool.tile([128, OC], f32, tag="ps", name=f"ps_{oc}_{t}")
                    for t in range(N_TC)
                ]
                # Seed each accumulation group with the rank-16 LoRA matmul.
                for t in range(N_TC):
                    nc.tensor.matmul(
                        ps_tiles[t][:],
                        lora1_sb[:, t * 128 : (t + 1) * 128],
                        bt_sb[:, osl],
                        start=True,
                        stop=False,
                    )
                for dc in range(N_DC):
                    wt = wpool.tile([128, OC], f32r, tag="wt")
                    nc.sync.dma_start(
                        out=wt[:], in_=WT[dc * 128 : (dc + 1) * 128, osl]
                    )
                    for t in range(N_TC):
                        nc.tensor.matmul(
                            ps_tiles[t][:],
                            xt_tiles[dc][:, t * 128 : (t + 1) * 128],
                            wt[:],
                            start=False,
                            stop=(dc == N_DC - 1),
                        )
                for t in range(N_TC):
                    o_sb = opool.tile([128, OC], f32, tag="osb")
                    nc.vector.tensor_tensor(
                        o_sb[:],
                        ps_tiles[t][:],
                        bias_sb[:, osl],
                        mybir.AluOpType.add,
                    )
                    nc.sync.dma_start(
                        out=out[t * 128 : (t + 1) * 128, osl], in_=o_sb[:]
                    )

    nc.compile()
    return nc


def _prepare_in_maps(x, W, b, lora_a, lora_b):
    import ml_dtypes

    WT = np.ascontiguousarray(W.T)  # [D_IN, D_OUT] fp32
    AT = np.ascontiguousarray(lora_a.T).astype(np.float32)  # [D_IN, R]
    # Fold the LoRA scaling (alpha/r = 2.0) into B^T; exact in bf16.
    BT = (np.ascontiguousarray(lora_b.T).astype(np.float32) * 2.0).astype(
        ml_dtypes.bfloat16
    )  # [R, D_OUT]
    bias = np.ascontiguousarray(
        np.broadcast_to(b.astype(np.float32), (128, D_OUT))
    )
    in_maps = []
    for c in range(N_CORES):
        xTc = np.ascontiguousarray(x[c * TPC : (c + 1) * TPC].T)
        in_maps.append(
            {"xT": xTc, "WT": WT, "AT": AT, "BT": BT, "bias": bias}
        )
    return in_maps


def run(inputs, trace=False, **trace_kwargs):
    """Run on hardware; returns (full_output, BassKernelResults)."""
    if "nc" not in _NC_CACHE:
        _NC_CACHE["nc"] = build_nc()
    nc = _NC_CACHE["nc"]
    in_maps = _prepare_in_maps(
        np.asarray(inputs["x"], dtype=np.float32),
        np.asarray(inputs["W"], dtype=np.float32),
        np.asarray(inputs["b"], dtype=np.float32),
        np.asarray(inputs["lora_a"]),
        np.asarray(inputs["lora_b"]),
    )
    res = run_bass_kernel_spmd(
        nc, in_maps, list(range(N_CORES)), trace=trace, **trace_kwargs
    )
    out = np.concatenate(
        [res.results[c]["out"] for c in range(N_CORES)], axis=0
    )
    return out.astype(np.float32), res


def kernel(**inputs):
    out, _ = run(inputs, trace=False)
    return out


if __name__ == "__main__":
    rng = np.random.default_rng(0)
    import ml_dtypes

    x = rng.standard_normal((T, D_IN), dtype=np.float32)
    W = rng.standard_normal((D_OUT, D_IN), dtype=np.float32) * 0.02
    b = rng.standard_normal((D_OUT,), dtype=np.float32) * 0.02
    la = (rng.standard_normal((R, D_IN), dtype=np.float32) * 0.02).astype(
        ml_dtypes.bfloat16
    )
    lb = (rng.standard_normal((D_OUT, R), dtype=np.float32) * 0.02).astype(
        ml_dtypes.bfloat16
    )
    got = kernel(x=x, W=W, b=b, lora_a=la, lora_b=lb)
    ref = (
        x @ W.T
        + b
        + 2.0
        * (
            (x.astype(ml_dtypes.bfloat16).astype(np.float32) @ la.astype(np.float32).T)
            @ lb.astype(np.float32).T
        )
    )
    err = np.abs(got - ref).max() / np.abs(ref).max()
    print("scale-relative max err:", err)
